# revision 1
# baseline (speedup 1.0000x reference)
"""CondConv3d kernel for 8 TRN2 NeuronCores.

Math: the reference einsum 'bi,eocdwh->bocdwh' shares no index between
routing_weights and weight, so it factorizes:
    eff_kernel[b] = (sum_i routing[b,i]) * (sum_e weight[e])
    eff_bias[b]   = (sum_i routing[b,i]) * (sum_e bias[e])
=> out[b] = conv3d(x[b], s_b * W_sum, pad=1) + s_b * bias_sum

Sharding: data-parallel over batch B=8, one sample per core. The
per-sample scalar s_b is folded into that core's weights/bias on host.

Per-core kernel (bf16 inputs, fp32 accumulate):
  - x is staged in SBUF in a zero-padded layout: per depth-slice "slot"
    of 68*66 bf16 per partition, content at rows 2..65, cols 0..63 of a
    68x66 grid (everything else zero).  Partition blocks hold kd-shifted
    copies: partitions [0,32)=x(d-1), [32,64)=x(d), [64,96)=x(d+1).
  - conv = 9 PSUM-accumulated matmuls (kh,kw taps via free-dim address
    shifts of +-66/+-1), contraction 96 = (kd, C_in).
  - 4 depth slices run concurrently via PE column tiling
    (tile_position=(0,32j)) so the PSUM/drain uses all 128 partitions.
  - drain: ScalarE/VectorE copy PSUM->SBUF with per-partition bias add,
    stripping the 2 pad columns; one contiguous DMA out per 4-slice group.
"""

import sys

if "/opt/trn_rl_repo" not in sys.path:
    sys.path.insert(0, "/opt/trn_rl_repo")

import numpy as np
import ml_dtypes

import concourse.bass as bass
import concourse.tile as tile
from concourse import bacc, mybir
from concourse.bass_utils import run_bass_kernel_spmd

# problem shape (hardcoded per contest rules)
B, CI, CO, D, H, W = 8, 32, 32, 16, 64, 64
K = 3
NCORES = 8

# padded slot layout
WP = 66                 # padded row width (64 valid + 2 zero cols)
SLOT_ROWS = 68          # 2 zero rows, 64 content rows, 2 zero rows
SLOT = SLOT_ROWS * WP   # 4488 elements per depth-slice per partition
Q0 = 2 * WP             # content base offset inside a slot
R = 9                   # ring depth (slots live per partition)

NSTEP = 9               # (kh, kw) taps
ROWS_PER_CHUNK = 7
CHUNK = ROWS_PER_CHUNK * WP  # 462 <= 512 (one PSUM bank)
# chunk start rows; last chunk overlaps (recomputes rows 57..62, drains row 63)
CHUNK_R0 = [0, 7, 14, 21, 28, 35, 42, 49, 56, 57]

F32 = mybir.dt.float32
BF16 = mybir.dt.bfloat16

_CACHE = {}


def _build_nc():
    # Bacc (vs raw Bass) runs the wait-fixup passes: an ISA instruction can
    # carry only 1 semaphore wait; Bacc spills extras to ldweights/events.
    nc = bacc.Bacc(None)
    x_d = nc.declare_dram_parameter("x", [CI, D, H * W], BF16, isOutput=False)
    w_d = nc.declare_dram_parameter("w", [96, NSTEP * CO], BF16, isOutput=False)
    b_d = nc.declare_dram_parameter("bias", [128, 1], F32, isOutput=False)
    o_d = nc.declare_dram_parameter("out", [CO, D, H * W], F32, isOutput=True)

    with tile.TileContext(nc) as tc:
        with (
            tc.tile_pool(name="const", bufs=1) as const,
            tc.tile_pool(name="outs", bufs=2) as outp,
            tc.tile_pool(name="psum", bufs=6, space="PSUM") as psump,
        ):
            xp = const.tile([96, R, SLOT], BF16)
            wsb = const.tile([96, NSTEP, CO], BF16)
            bsb = const.tile([128, 1], F32)

            nc.sync.dma_start(
                out=wsb[:, :, :],
                in_=w_d[:].rearrange("p (s o) -> p s o", s=NSTEP),
            )
            nc.sync.dma_start(out=bsb[:, :], in_=b_d[:])

            # zero-init the whole ring (pads stay zero forever)
            nc.vector.memset(xp[:, :, :], 0.0)

            def fill(s):
                """Load x depth-slice s and place its three kd-copies."""
                slot = s % R
                dst = xp[32:64, slot, :].rearrange(
                    "p (h w) -> p h w", h=SLOT_ROWS
                )[:, 2:66, 0:64]
                src = x_d[:, s, :].rearrange("p (h w) -> p h w", h=H)
                nc.sync.dma_start(out=dst, in_=src)
                if s > 0:
                    prev = (s - 1) % R
                    # b0(slot s) = x(s-1): pull from previous slot's center
                    nc.sync.dma_start(
                        out=xp[0:32, slot, :], in_=xp[32:64, prev, :]
                    )
                    # b2(slot s-1) = x(s): push back
                    nc.sync.dma_start(
                        out=xp[64:96, prev, :], in_=xp[32:64, slot, :]
                    )
                if s == D - 1:
                    # slot for d=15 was last written as b2=x(7) in ring era 0;
                    # d=15 needs b2=x(16)=0
                    nc.vector.memset(xp[64:96, slot, :], 0.0)

            def compute_group(g):
                ob = outp.tile([128, H * W], F32)
                for ci_, r0 in enumerate(CHUNK_R0):
                    # full 512-float bank so every tile is bank-aligned
                    ps_full = psump.tile([128, 512], F32)
                    ps = ps_full[:, 0:CHUNK]
                    for s in range(NSTEP):
                        kh, kw = s // 3, s % 3
                        off = (kh - 1) * WP + (kw - 1)
                        for j in range(4):
                            d = 4 * g + j
                            slot = d % R
                            base = Q0 + r0 * WP + off
                            rhs = xp[0:96, slot, base : base + CHUNK]
                            nc.tensor.matmul(
                                out=ps[32 * j : 32 * j + 32, :],
                                lhsT=wsb[0:96, s, :],
                                rhs=rhs,
                                start=(s == 0),
                                stop=(s == NSTEP - 1),
                                tile_position=(0, 32 * j),
                                # sim's group tracker is bank-coarse; the
                                # 4 col-tiles run disjoint partition ranges
                                skip_group_check=True,
                            )
                    # drain PSUM -> SBUF, strip pad cols, add bias
                    ps3 = ps[:, :].rearrange(
                        "p (h w) -> p h w", h=ROWS_PER_CHUNK
                    )
                    if r0 == 57:
                        src3 = ps3[:, 6:7, 0:64]
                        dst3 = ob[:, 63 * 64 : 64 * 64].rearrange(
                            "p (h w) -> p h w", h=1
                        )
                    else:
                        src3 = ps3[:, :, 0:64]
                        dst3 = ob[:, r0 * 64 : (r0 + 7) * 64].rearrange(
                            "p (h w) -> p h w", h=ROWS_PER_CHUNK
                        )
                    if ci_ % 2 == 0:
                        nc.scalar.activation(
                            out=dst3,
                            in_=src3,
                            func=mybir.ActivationFunctionType.Identity,
                            bias=bsb[:, :],
                            scale=1.0,
                        )
                    else:
                        nc.vector.tensor_scalar_add(dst3, src3, bsb[:, :])
                # out partitions are (j, o); DRAM wants [o][d][hw]
                dst = bass.AP(
                    tensor=o_d,
                    offset=4 * g * (H * W),
                    ap=[
                        [H * W, 4],          # j (depth slice within group)
                        [D * H * W, CO],     # o (channel)
                        [1, H * W],
                    ],
                )
                nc.sync.dma_start(out=dst, in_=ob[:, :])

            for s in range(5):
                fill(s)
            for g in range(4):
                for s in range(4 * g + 5, min(4 * g + 9, D)):
                    fill(s)
                compute_group(g)

    nc.finalize()  # Bacc: runs wait-spill + register allocation passes
    return nc


def _get_nc():
    if "nc" not in _CACHE:
        _CACHE["nc"] = _build_nc()
    return _CACHE["nc"]


def kernel(x, routing_weights, weight, bias):
    x = np.asarray(x, dtype=np.float32)
    routing_weights = np.asarray(routing_weights, dtype=np.float32)
    weight = np.asarray(weight, dtype=np.float32)
    bias = np.asarray(bias, dtype=np.float32)

    s = routing_weights.sum(axis=1)          # [B]
    w_sum = weight.sum(axis=0)               # [CO, CI, K, K, K]
    b_sum = bias.sum(axis=0)                 # [CO]

    # lhsT layout: [p=(kd,ci), (kh,kw), o]
    wt = np.transpose(w_sum, (2, 1, 3, 4, 0)).reshape(96, NSTEP * CO)

    in_maps = []
    for b in range(B):
        wb = (s[b] * wt).astype(ml_dtypes.bfloat16)
        bb = np.tile(s[b] * b_sum, 4).reshape(128, 1).astype(np.float32)
        in_maps.append(
            {
                "x": np.ascontiguousarray(
                    x[b].reshape(CI, D, H * W).astype(ml_dtypes.bfloat16)
                ),
                "w": np.ascontiguousarray(wb),
                "bias": bb,
            }
        )

    nc = _get_nc()
    _CACHE["last_in_maps"] = in_maps
    res = run_bass_kernel_spmd(nc, in_maps, list(range(NCORES)))
    _CACHE["last_result"] = res
    out = np.stack(
        [
            np.asarray(res.results[b]["out"], dtype=np.float32).reshape(
                CO, D, H, W
            )
            for b in range(B)
        ]
    )
    return out



# revision 2
# speedup vs baseline: 1.8484x; 1.8484x over previous
"""CondConv3d kernel for 8 TRN2 NeuronCores.

Math: the reference einsum 'bi,eocdwh->bocdwh' shares no index between
routing_weights and weight, so it factorizes:
    eff_kernel[b] = (sum_i routing[b,i]) * (sum_e weight[e])
    eff_bias[b]   = (sum_i routing[b,i]) * (sum_e bias[e])
=> out[b] = conv3d(x[b], s_b * W_sum, pad=1) + s_b * bias_sum

Sharding: data-parallel over batch B=8, one sample per core. The
per-sample scalar s_b is folded into that core's weights/bias on host.

Per-core kernel (bf16 in/out, fp32 PSUM accumulate):
  - x is padded on the HOST to 66x66 per depth slice (content at rows/
    cols 1..64, zeros elsewhere) plus a leading all-zero slice. Device
    DMAs are therefore large contiguous copies (8.7KB per partition per
    slice) instead of per-row 128B packets.
  - SBUF holds the whole tensor as [96, 16*4356+66] bf16: partition
    groups [0,32)/[32,64)/[64,96) hold kd-shifted copies x(d-1)/x(d)/
    x(d+1), each loaded straight from HBM (the zero slice supplies the
    depth pads).
  - conv = 9 PSUM-accumulated matmuls (kh,kw taps via free-dim address
    shifts of +-66/+-1), contraction 96 = (kd, C_in).
  - 4 depth slices run concurrently via PE column tiling
    (tile_position=(0,32j)) so the PSUM/drain uses all 128 partitions.
  - drain: ScalarE/VectorE copy PSUM->SBUF bf16 with per-partition bias
    add, stripping pad columns; one contiguous DMA out per 4-slice group.
"""

import sys

if "/opt/trn_rl_repo" not in sys.path:
    sys.path.insert(0, "/opt/trn_rl_repo")

import numpy as np
import ml_dtypes

import concourse.bass as bass
import concourse.tile as tile
from concourse import bacc, mybir
from concourse.bass_utils import run_bass_kernel_spmd

# problem shape (hardcoded per contest rules)
B, CI, CO, D, H, W = 8, 32, 32, 16, 64, 64
K = 3
NCORES = 8

# host-padded slice layout: 66x66, content at [1:65, 1:65]
WP = 66
SLOT = WP * WP          # 4356 elements per depth slice per partition
NSLICE = D + 1          # leading all-zero slice at index 0
Q0 = WP + 1             # content (row 1, col 1) offset inside a slot
GUARD = WP              # tail guard so the last chunk's +67 tap stays in-tile

NSTEP = 9               # (kh, kw) taps
ROWS_PER_CHUNK = 7
CHUNK = ROWS_PER_CHUNK * WP  # 462 <= 512 (one PSUM bank)
# chunk start rows; last chunk overlaps (recomputes rows 57..62, drains row 63)
CHUNK_R0 = [0, 7, 14, 21, 28, 35, 42, 49, 56, 57]

F32 = mybir.dt.float32
BF16 = mybir.dt.bfloat16

_CACHE = {}


def _build_nc():
    # Bacc (vs raw Bass) runs the wait-fixup passes: an ISA instruction can
    # carry only 1 semaphore wait; Bacc spills extras to ldweights/events.
    nc = bacc.Bacc(None)
    x_d = nc.declare_dram_parameter("x", [CI, NSLICE, SLOT], BF16, isOutput=False)
    w_d = nc.declare_dram_parameter("w", [96, NSTEP * CO], BF16, isOutput=False)
    b_d = nc.declare_dram_parameter("bias", [128, 1], F32, isOutput=False)
    o_d = nc.declare_dram_parameter("out", [CO, D, H * W], BF16, isOutput=True)

    with tile.TileContext(nc) as tc:
        with (
            tc.tile_pool(name="const", bufs=1) as const,
            tc.tile_pool(name="outs", bufs=2) as outp,
            tc.tile_pool(name="psum", bufs=6, space="PSUM") as psump,
        ):
            xp = const.tile([96, D * SLOT + GUARD], BF16)
            wsb = const.tile([96, NSTEP, CO], BF16)
            bsb = const.tile([128, 1], F32)

            nc.sync.dma_start(
                out=wsb[:, :, :],
                in_=w_d[:].rearrange("p (s o) -> p s o", s=NSTEP),
            )
            nc.sync.dma_start(out=bsb[:, :], in_=b_d[:])
            nc.vector.memset(xp[:, D * SLOT :], 0.0)

            def fill(d):
                """Load slot d of all three kd-shifted partition groups.

                x_d slice k holds padded x[k-1] (k=0 is the zero slice), so
                group g's slot d wants x[d+g-1] = x_d slice d+g.
                """
                dst = xp[:, d * SLOT : (d + 1) * SLOT]
                nc.sync.dma_start(out=dst[32:64], in_=x_d[:, d + 1, :])
                nc.sync.dma_start(out=dst[0:32], in_=x_d[:, d, :])
                src2 = d + 2 if d + 2 <= D else 0
                nc.sync.dma_start(out=dst[64:96], in_=x_d[:, src2, :])

            def compute_group(g):
                ob = outp.tile([128, H * W], BF16)
                for ci_, r0 in enumerate(CHUNK_R0):
                    # full 512-float bank so every tile is bank-aligned
                    ps_full = psump.tile([128, 512], F32)
                    ps = ps_full[:, 0:CHUNK]
                    for s in range(NSTEP):
                        kh, kw = s // 3, s % 3
                        off = (kh - 1) * WP + (kw - 1)
                        for j in range(4):
                            d = 4 * g + j
                            base = d * SLOT + Q0 + r0 * WP + off
                            rhs = xp[0:96, base : base + CHUNK]
                            nc.tensor.matmul(
                                out=ps[32 * j : 32 * j + 32, :],
                                lhsT=wsb[0:96, s, :],
                                rhs=rhs,
                                start=(s == 0),
                                stop=(s == NSTEP - 1),
                                tile_position=(0, 32 * j),
                                # sim's group tracker is bank-coarse; the
                                # 4 col-tiles run disjoint partition ranges
                                skip_group_check=True,
                            )
                    # drain PSUM -> SBUF, strip pad cols, add bias
                    ps3 = ps[:, :].rearrange(
                        "p (h w) -> p h w", h=ROWS_PER_CHUNK
                    )
                    if r0 == 57:
                        src3 = ps3[:, 6:7, 0:64]
                        dst3 = ob[:, 63 * 64 : 64 * 64].rearrange(
                            "p (h w) -> p h w", h=1
                        )
                    else:
                        src3 = ps3[:, :, 0:64]
                        dst3 = ob[:, r0 * 64 : (r0 + 7) * 64].rearrange(
                            "p (h w) -> p h w", h=ROWS_PER_CHUNK
                        )
                    if ci_ % 2 == 0:
                        nc.scalar.activation(
                            out=dst3,
                            in_=src3,
                            func=mybir.ActivationFunctionType.Identity,
                            bias=bsb[:, :],
                            scale=1.0,
                        )
                    else:
                        nc.vector.tensor_scalar_add(dst3, src3, bsb[:, :])
                # out partitions are (j, o); DRAM wants [o][d][hw]
                dst = bass.AP(
                    tensor=o_d,
                    offset=4 * g * (H * W),
                    ap=[
                        [H * W, 4],          # j (depth slice within group)
                        [D * H * W, CO],     # o (channel)
                        [1, H * W],
                    ],
                )
                nc.sync.dma_start(out=dst, in_=ob[:, :])

            for d in range(4):
                fill(d)
            for g in range(4):
                for d in range(4 * g + 4, min(4 * g + 8, D)):
                    fill(d)
                compute_group(g)

    nc.finalize()  # Bacc: runs wait-spill + register allocation passes
    return nc


def _get_nc():
    if "nc" not in _CACHE:
        _CACHE["nc"] = _build_nc()
    return _CACHE["nc"]


def _host_prep(x, routing_weights, weight, bias):
    """Build the per-core input maps (one batch sample per core)."""
    x = np.asarray(x, dtype=np.float32)
    routing_weights = np.asarray(routing_weights, dtype=np.float32)
    weight = np.asarray(weight, dtype=np.float32)
    bias = np.asarray(bias, dtype=np.float32)

    s = routing_weights.sum(axis=1)          # [B]
    w_sum = weight.sum(axis=0)               # [CO, CI, K, K, K]
    b_sum = bias.sum(axis=0)                 # [CO]

    # lhsT layout: [p=(kd,ci), (kh,kw), o]
    wt = np.transpose(w_sum, (2, 1, 3, 4, 0)).reshape(96, NSTEP * CO)

    # host-side zero padding: slice k = padded x[k-1], slice 0 all-zero
    xpad = np.zeros((B, CI, NSLICE, WP, WP), dtype=np.float32)
    xpad[:, :, 1:, 1:65, 1:65] = x.reshape(B, CI, D, H, W)
    xpad = xpad.reshape(B, CI, NSLICE, SLOT)

    in_maps = []
    for b in range(B):
        wb = (s[b] * wt).astype(ml_dtypes.bfloat16)
        bb = np.tile(s[b] * b_sum, 4).reshape(128, 1).astype(np.float32)
        in_maps.append(
            {
                "x": np.ascontiguousarray(xpad[b].astype(ml_dtypes.bfloat16)),
                "w": np.ascontiguousarray(wb),
                "bias": bb,
            }
        )
    return in_maps


def kernel(x, routing_weights, weight, bias):
    in_maps = _host_prep(x, routing_weights, weight, bias)
    nc = _get_nc()
    _CACHE["last_in_maps"] = in_maps
    res = run_bass_kernel_spmd(nc, in_maps, list(range(NCORES)))
    _CACHE["last_result"] = res
    out = np.stack(
        [
            np.asarray(res.results[b]["out"]).astype(np.float32).reshape(
                CO, D, H, W
            )
            for b in range(B)
        ]
    )
    return out


# revision 6
# speedup vs baseline: 1.8575x; 1.0049x over previous
"""CondConv3d kernel for 8 TRN2 NeuronCores.

Math: the reference einsum 'bi,eocdwh->bocdwh' shares no index between
routing_weights and weight, so it factorizes:
    eff_kernel[b] = (sum_i routing[b,i]) * (sum_e weight[e])
    eff_bias[b]   = (sum_i routing[b,i]) * (sum_e bias[e])
=> out[b] = conv3d(x[b], s_b * W_sum, pad=1) + s_b * bias_sum

Sharding: data-parallel over batch B=8, one sample per core. The
per-sample scalar s_b is folded into that core's weights/bias on host.

Per-core kernel (bf16 in/out, fp32 PSUM accumulate):
  - x is padded on the HOST to 66x66 per depth slice (content at rows/
    cols 1..64, zeros elsewhere) plus a leading all-zero slice. Device
    DMAs are therefore large contiguous copies (8.7KB per partition per
    slice) instead of per-row 128B packets.
  - SBUF holds the whole tensor as [96, 16*4356+66] bf16: partition
    groups [0,32)/[32,64)/[64,96) hold kd-shifted copies x(d-1)/x(d)/
    x(d+1), each loaded straight from HBM (the zero slice supplies the
    depth pads).
  - conv = 9 PSUM-accumulated matmuls (kh,kw taps via free-dim address
    shifts of +-66/+-1), contraction 96 = (kd, C_in).
  - 4 depth slices run concurrently via PE column tiling
    (tile_position=(0,32j)) so the PSUM/drain uses all 128 partitions.
  - drain: ScalarE/VectorE copy PSUM->SBUF bf16 with per-partition bias
    add, stripping pad columns; one contiguous DMA out per 4-slice group.
"""

import sys

if "/opt/trn_rl_repo" not in sys.path:
    sys.path.insert(0, "/opt/trn_rl_repo")

import numpy as np
import ml_dtypes

import concourse.bass as bass
import concourse.tile as tile
from concourse import bacc, mybir
from concourse.bass_utils import run_bass_kernel_spmd

# problem shape (hardcoded per contest rules)
B, CI, CO, D, H, W = 8, 32, 32, 16, 64, 64
K = 3
NCORES = 8

# host-padded slice layout: 66x66, content at [1:65, 1:65]
WP = 66
SLOT = WP * WP          # 4356 elements per depth slice per partition
NSLICE = D + 1          # leading all-zero slice at index 0
Q0 = WP + 1             # content (row 1, col 1) offset inside a slot
GUARD = WP              # tail guard so the last chunk's +67 tap stays in-tile

NSTEP = 9               # (kh, kw) taps
ROWS_PER_CHUNK = 7
CHUNK = ROWS_PER_CHUNK * WP  # 462 <= 512 (one PSUM bank)
# chunk start rows; last chunk overlaps (recomputes rows 57..62, drains row 63)
CHUNK_R0 = [0, 7, 14, 21, 28, 35, 42, 49, 56, 57]

F32 = mybir.dt.float32
BF16 = mybir.dt.bfloat16

_CACHE = {}


def _build_nc():
    # Bacc (vs raw Bass) runs the wait-fixup passes: an ISA instruction can
    # carry only 1 semaphore wait; Bacc spills extras to ldweights/events.
    nc = bacc.Bacc(None)
    x_d = nc.declare_dram_parameter("x", [CI, NSLICE, SLOT], BF16, isOutput=False)
    w_d = nc.declare_dram_parameter("w", [96, NSTEP * CO], BF16, isOutput=False)
    b_d = nc.declare_dram_parameter("bias", [128, 1], F32, isOutput=False)
    o_d = nc.declare_dram_parameter("out", [CO, D, H * W], BF16, isOutput=True)

    with tile.TileContext(nc) as tc:
        with (
            tc.tile_pool(name="const", bufs=1) as const,
            tc.tile_pool(name="outs", bufs=2) as outp,
            tc.tile_pool(name="psum", bufs=8, space="PSUM") as psump,
        ):
            xp = const.tile([96, D * SLOT + GUARD], BF16)
            wsb = const.tile([96, NSTEP, CO], BF16)
            bsb = const.tile([128, 1], F32)

            nc.sync.dma_start(
                out=wsb[:, :, :],
                in_=w_d[:].rearrange("p (s o) -> p s o", s=NSTEP),
            )
            nc.sync.dma_start(out=bsb[:, :], in_=b_d[:])
            nc.vector.memset(xp[:, D * SLOT :], 0.0)

            def fill(q):
                """Load slots 4q..4q+3 of all three kd-shifted groups.

                x_d slice k holds padded x[k-1] (k=0 is the zero slice), so
                group g's slot d wants x[d+g-1] = x_d slice d+g. Batching 4
                slices per DMA keeps the sync engine's descriptor-dispatch
                cost off the critical path.
                """
                d = 4 * q
                dst = xp[:, d * SLOT : (d + 4) * SLOT]
                nc.sync.dma_start(out=dst[32:64], in_=x_d[:, d + 1 : d + 5, :])
                nc.sync.dma_start(out=dst[0:32], in_=x_d[:, d : d + 4, :])
                if q < 3:
                    nc.sync.dma_start(
                        out=dst[64:96], in_=x_d[:, d + 2 : d + 6, :]
                    )
                else:
                    # d=12..14 -> x_d 14..16; d=15 wants x[16]=0 = x_d slice 0
                    nc.sync.dma_start(
                        out=xp[64:96, d * SLOT : (d + 3) * SLOT],
                        in_=x_d[:, 14:17, :],
                    )
                    nc.sync.dma_start(
                        out=xp[64:96, (d + 3) * SLOT : (d + 4) * SLOT],
                        in_=x_d[:, 0, :],
                    )

            def compute_group(g):
                ob = outp.tile([128, H * W], BF16)
                for ci_, r0 in enumerate(CHUNK_R0):
                    # full 512-float bank so every tile is bank-aligned
                    ps_full = psump.tile([128, 512], F32)
                    ps = ps_full[:, 0:CHUNK]
                    for s in range(NSTEP):
                        kh, kw = s // 3, s % 3
                        off = (kh - 1) * WP + (kw - 1)
                        for j in range(4):
                            d = 4 * g + j
                            base = d * SLOT + Q0 + r0 * WP + off
                            rhs = xp[0:96, base : base + CHUNK]
                            nc.tensor.matmul(
                                out=ps[32 * j : 32 * j + 32, :],
                                lhsT=wsb[0:96, s, :],
                                rhs=rhs,
                                start=(s == 0),
                                stop=(s == NSTEP - 1),
                                tile_position=(0, 32 * j),
                                # sim's group tracker is bank-coarse; the
                                # 4 col-tiles run disjoint partition ranges
                                skip_group_check=True,
                            )
                    # drain PSUM -> SBUF, strip pad cols, add bias
                    ps3 = ps[:, :].rearrange(
                        "p (h w) -> p h w", h=ROWS_PER_CHUNK
                    )
                    if r0 == 57:
                        src3 = ps3[:, 6:7, 0:64]
                        dst3 = ob[:, 63 * 64 : 64 * 64].rearrange(
                            "p (h w) -> p h w", h=1
                        )
                    else:
                        src3 = ps3[:, :, 0:64]
                        dst3 = ob[:, r0 * 64 : (r0 + 7) * 64].rearrange(
                            "p (h w) -> p h w", h=ROWS_PER_CHUNK
                        )
                    if ci_ % 2 == 0:
                        nc.scalar.activation(
                            out=dst3,
                            in_=src3,
                            func=mybir.ActivationFunctionType.Identity,
                            bias=bsb[:, :],
                            scale=1.0,
                        )
                    else:
                        nc.vector.tensor_scalar_add(dst3, src3, bsb[:, :])
                    # stream finished rows out early so the final DMA isn't
                    # a 1MB tail after the last matmul
                    if ci_ in (3, 7, 9):
                        lo = {3: 0, 7: 28 * 64, 9: 56 * 64}[ci_]
                        hi = {3: 28 * 64, 7: 56 * 64, 9: 64 * 64}[ci_]
                        dst = bass.AP(
                            tensor=o_d,
                            offset=4 * g * (H * W) + lo,
                            ap=[
                                [H * W, 4],       # j (slice within group)
                                [D * H * W, CO],  # o (channel)
                                [1, hi - lo],
                            ],
                        )
                        nc.sync.dma_start(out=dst, in_=ob[:, lo:hi])

            fill(0)
            for g in range(4):
                if g + 1 < 4:
                    fill(g + 1)
                compute_group(g)

    nc.finalize()  # Bacc: runs wait-spill + register allocation passes
    return nc


def _get_nc():
    if "nc" not in _CACHE:
        _CACHE["nc"] = _build_nc()
    return _CACHE["nc"]


def _host_prep(x, routing_weights, weight, bias):
    """Build the per-core input maps (one batch sample per core)."""
    x = np.asarray(x, dtype=np.float32)
    routing_weights = np.asarray(routing_weights, dtype=np.float32)
    weight = np.asarray(weight, dtype=np.float32)
    bias = np.asarray(bias, dtype=np.float32)

    s = routing_weights.sum(axis=1)          # [B]
    w_sum = weight.sum(axis=0)               # [CO, CI, K, K, K]
    b_sum = bias.sum(axis=0)                 # [CO]

    # lhsT layout: [p=(kd,ci), (kh,kw), o]
    wt = np.transpose(w_sum, (2, 1, 3, 4, 0)).reshape(96, NSTEP * CO)

    # host-side zero padding: slice k = padded x[k-1], slice 0 all-zero
    xpad = np.zeros((B, CI, NSLICE, WP, WP), dtype=np.float32)
    xpad[:, :, 1:, 1:65, 1:65] = x.reshape(B, CI, D, H, W)
    xpad = xpad.reshape(B, CI, NSLICE, SLOT)

    in_maps = []
    for b in range(B):
        wb = (s[b] * wt).astype(ml_dtypes.bfloat16)
        bb = np.tile(s[b] * b_sum, 4).reshape(128, 1).astype(np.float32)
        in_maps.append(
            {
                "x": np.ascontiguousarray(xpad[b].astype(ml_dtypes.bfloat16)),
                "w": np.ascontiguousarray(wb),
                "bias": bb,
            }
        )
    return in_maps


def kernel(x, routing_weights, weight, bias):
    in_maps = _host_prep(x, routing_weights, weight, bias)
    nc = _get_nc()
    _CACHE["last_in_maps"] = in_maps
    res = run_bass_kernel_spmd(nc, in_maps, list(range(NCORES)))
    _CACHE["last_result"] = res
    out = np.stack(
        [
            np.asarray(res.results[b]["out"]).astype(np.float32).reshape(
                CO, D, H, W
            )
            for b in range(B)
        ]
    )
    return out


# revision 7
# speedup vs baseline: 2.0187x; 1.0868x over previous
"""CondConv3d kernel for 8 TRN2 NeuronCores (v2: unpadded 64-wide rows).

Math: the reference einsum 'bi,eocdwh->bocdwh' shares no index between
routing_weights and weight, so it factorizes:
    eff_kernel[b] = (sum_i routing[b,i]) * (sum_e weight[e])
    eff_bias[b]   = (sum_i routing[b,i]) * (sum_e bias[e])
=> out[b] = conv3d(x[b], s_b * W_sum, pad=1) + s_b * bias_sum

Sharding: data-parallel over batch B=8, one sample per core. The
per-sample scalar s_b is folded into that core's weights/bias on host.

Per-core kernel (bf16 in/out, fp32 PSUM accumulate):
  - x lives in SBUF UNPADDED: [96, 16*4096] bf16, partition groups
    [0,32)/[32,64)/[64,96) hold kd-shifted copies x(d-1)/x(d)/x(d+1),
    each loaded straight from HBM (a leading all-zero slice in the
    DRAM tensor supplies the depth pads). All DMAs are large contiguous
    copies (4 slices = 32KB per partition per transfer).
  - conv = 9 PSUM-accumulated matmuls per 512-output chunk; the kh/kw
    taps are free-dim address shifts of +-64/+-1. Boundary zero-padding
    is realized by RESTRICTING the matmul APs instead of padding data:
    kw=0 taps write out cols 1..63 only, kw=2 cols 0..62 (2-D APs with
    row stride 64); kh=0 skips out row 0 of each slice (chunk 0), kh=2
    skips row 63 (chunk 7). The first tap (kh=1,kw=1) is full-512 and
    carries the PSUM start flag.
  - 4 depth slices run concurrently via PE column tiling
    (tile_position=(0,32j)) so the PSUM/drain uses all 128 partitions.
  - drain: ScalarE/VectorE copy full contiguous [128,512] PSUM->SBUF
    bf16 with per-partition bias add; output streams out in two DMAs
    per 4-slice group so there is no large DMA tail.
"""

import sys

if "/opt/trn_rl_repo" not in sys.path:
    sys.path.insert(0, "/opt/trn_rl_repo")

import numpy as np
import ml_dtypes

import concourse.bass as bass
import concourse.tile as tile
from concourse import bacc, mybir
from concourse.bass_utils import run_bass_kernel_spmd

# problem shape (hardcoded per contest rules)
B, CI, CO, D, H, W = 8, 32, 32, 16, 64, 64
K = 3
NCORES = 8

SLOT = H * W            # 4096 elements per depth slice per partition
NSLICE = D + 1          # leading all-zero slice at index 0
VOL0 = 64               # front guard (never read; belt-and-suspenders)
NSTEP = 9
NCHUNK = 8              # 8 chunks of 512 = one 64x64 slice

# tap order: the full-rectangle (kh=1,kw=1) tap goes first so its
# start=True initializes every PSUM position of the chunk.
TAPS = [(1, 1), (1, 0), (1, 2), (0, 0), (0, 1), (0, 2), (2, 0), (2, 1), (2, 2)]

F32 = mybir.dt.float32
BF16 = mybir.dt.bfloat16

_CACHE = {}


def _build_nc():
    # Bacc (vs raw Bass) runs the wait-fixup passes: an ISA instruction can
    # carry only 1 semaphore wait; Bacc spills extras to ldweights/events.
    nc = bacc.Bacc(None)
    x_d = nc.declare_dram_parameter("x", [CI, NSLICE, SLOT], BF16, isOutput=False)
    w_d = nc.declare_dram_parameter("w", [96, NSTEP * CO], BF16, isOutput=False)
    b_d = nc.declare_dram_parameter("bias", [128, 1], F32, isOutput=False)
    o_d = nc.declare_dram_parameter("out", [CO, D, H * W], BF16, isOutput=True)

    with tile.TileContext(nc) as tc:
        with (
            tc.tile_pool(name="const", bufs=1) as const,
            tc.tile_pool(name="outs", bufs=2) as outp,
            tc.tile_pool(name="psum", bufs=8, space="PSUM") as psump,
        ):
            xp = const.tile([96, VOL0 + D * SLOT + 64], BF16)
            wsb = const.tile([96, NSTEP, CO], BF16)
            bsb = const.tile([128, 1], F32)

            nc.sync.dma_start(
                out=wsb[:, :, :],
                in_=w_d[:].rearrange("p (s o) -> p s o", s=NSTEP),
            )
            nc.sync.dma_start(out=bsb[:, :], in_=b_d[:])
            nc.vector.memset(xp[:, :VOL0], 0.0)
            nc.vector.memset(xp[:, VOL0 + D * SLOT :], 0.0)

            def fill(q):
                """Load slots 4q..4q+3 of all three kd-shifted groups.

                x_d slice k holds x[k-1] (k=0 is the zero slice), so group
                g's slot d wants x[d+g-1] = x_d slice d+g. Batching 4 slices
                per DMA keeps the sync engine's descriptor-dispatch cost off
                the critical path.
                """
                d = 4 * q
                o0 = VOL0 + d * SLOT
                dst = xp[:, o0 : o0 + 4 * SLOT]
                nc.sync.dma_start(out=dst[32:64], in_=x_d[:, d + 1 : d + 5, :])
                nc.sync.dma_start(out=dst[0:32], in_=x_d[:, d : d + 4, :])
                if q < 3:
                    nc.sync.dma_start(
                        out=dst[64:96], in_=x_d[:, d + 2 : d + 6, :]
                    )
                else:
                    # d=12..14 -> x_d 14..16; d=15 wants x[16]=0 = x_d slice 0
                    nc.sync.dma_start(
                        out=xp[64:96, o0 : o0 + 3 * SLOT],
                        in_=x_d[:, 14:17, :],
                    )
                    nc.sync.dma_start(
                        out=xp[64:96, o0 + 3 * SLOT : o0 + 4 * SLOT],
                        in_=x_d[:, 0, :],
                    )

            def compute_group(g):
                ob = outp.tile([128, H * W], BF16)
                for c8 in range(NCHUNK):
                    # one full PSUM bank = 8 output rows of 64
                    ps = psump.tile([128, 512], F32)
                    ps3 = ps[:, :].rearrange("p (h w) -> p h w", h=8)
                    for t, (kh, kw) in enumerate(TAPS):
                        # slice-edge rows whose kh tap would cross into the
                        # neighboring depth slice are simply not written
                        r0, r1 = 0, 8
                        if kh == 0 and c8 == 0:
                            r0 = 1
                        if kh == 2 and c8 == NCHUNK - 1:
                            r1 = 7
                        for j in range(4):
                            d = 4 * g + j
                            a = VOL0 + d * SLOT + c8 * 512 + (kh - 1) * 64
                            band = slice(32 * j, 32 * j + 32)
                            if kw == 1 and r0 == 0 and r1 == 8:
                                out_ap = ps[band, :]
                                rhs = xp[0:96, a : a + 512]
                            else:
                                v3 = xp[0:96, a : a + 512].rearrange(
                                    "p (h w) -> p h w", h=8
                                )
                                if kw == 1:
                                    out_ap = ps3[band, r0:r1, :]
                                    rhs = v3[:, r0:r1, :]
                                elif kw == 0:
                                    out_ap = ps3[band, r0:r1, 1:64]
                                    rhs = v3[:, r0:r1, 0:63]
                                else:  # kw == 2
                                    out_ap = ps3[band, r0:r1, 0:63]
                                    rhs = v3[:, r0:r1, 1:64]
                            nc.tensor.matmul(
                                out=out_ap,
                                lhsT=wsb[0:96, 3 * kh + kw, :],
                                rhs=rhs,
                                start=(t == 0),
                                stop=(t == NSTEP - 1),
                                tile_position=(0, 32 * j),
                                # sim's group tracker is bank-coarse; the
                                # 4 col-tiles run disjoint partition ranges
                                skip_group_check=True,
                            )
                    # drain PSUM -> SBUF bf16 with bias add (contiguous 512)
                    dst3 = ob[:, c8 * 512 : (c8 + 1) * 512]
                    if c8 % 2 == 0:
                        nc.scalar.activation(
                            out=dst3,
                            in_=ps[:, :],
                            func=mybir.ActivationFunctionType.Identity,
                            bias=bsb[:, :],
                            scale=1.0,
                        )
                    else:
                        nc.vector.tensor_scalar_add(dst3, ps[:, :], bsb[:, :])
                    # stream finished halves out so there is no big DMA tail
                    if c8 in (3, 7):
                        lo = 0 if c8 == 3 else 2048
                        hi = lo + 2048
                        dst = bass.AP(
                            tensor=o_d,
                            offset=4 * g * (H * W) + lo,
                            ap=[
                                [H * W, 4],       # j (slice within group)
                                [D * H * W, CO],  # o (channel)
                                [1, hi - lo],
                            ],
                        )
                        nc.sync.dma_start(out=dst, in_=ob[:, lo:hi])

            fill(0)
            for g in range(4):
                if g + 1 < 4:
                    fill(g + 1)
                compute_group(g)

    nc.finalize()  # Bacc: runs wait-spill + register allocation passes
    return nc


def _get_nc():
    if "nc" not in _CACHE:
        _CACHE["nc"] = _build_nc()
    return _CACHE["nc"]


def _host_prep(x, routing_weights, weight, bias):
    """Build the per-core input maps (one batch sample per core)."""
    x = np.asarray(x, dtype=np.float32)
    routing_weights = np.asarray(routing_weights, dtype=np.float32)
    weight = np.asarray(weight, dtype=np.float32)
    bias = np.asarray(bias, dtype=np.float32)

    s = routing_weights.sum(axis=1)          # [B]
    w_sum = weight.sum(axis=0)               # [CO, CI, K, K, K]
    b_sum = bias.sum(axis=0)                 # [CO]

    # lhsT layout: [p=(kd,ci), (kh,kw), o]
    wt = np.transpose(w_sum, (2, 1, 3, 4, 0)).reshape(96, NSTEP * CO)

    # slice 0 = zeros (depth pad); slice k = x[k-1]; no spatial padding
    xz = np.zeros((B, CI, NSLICE, SLOT), dtype=np.float32)
    xz[:, :, 1:, :] = x.reshape(B, CI, D, SLOT)

    in_maps = []
    for b in range(B):
        wb = (s[b] * wt).astype(ml_dtypes.bfloat16)
        bb = np.tile(s[b] * b_sum, 4).reshape(128, 1).astype(np.float32)
        in_maps.append(
            {
                "x": np.ascontiguousarray(xz[b].astype(ml_dtypes.bfloat16)),
                "w": np.ascontiguousarray(wb),
                "bias": bb,
            }
        )
    return in_maps


def kernel(x, routing_weights, weight, bias):
    in_maps = _host_prep(x, routing_weights, weight, bias)
    nc = _get_nc()
    _CACHE["last_in_maps"] = in_maps
    res = run_bass_kernel_spmd(nc, in_maps, list(range(NCORES)))
    _CACHE["last_result"] = res
    out = np.stack(
        [
            np.asarray(res.results[b]["out"]).astype(np.float32).reshape(
                CO, D, H, W
            )
            for b in range(B)
        ]
    )
    return out
